# revision 5
# baseline (speedup 1.0000x reference)
"""Trainium2 Bass kernel for nn_CustomGate: y = (I_64 (x) M (x) I_64) @ x.

Math: viewing x as (a=64, j=64, r=64, b=128), the gate is
    y[a,i,r,b] = sum_j M[i,j] * x[a,j,r,b]      (complex, M is 64x64)

Complex arithmetic is folded into one real 128x128 stationary weight
    W = [[Mr^T,  Mi^T ],
         [-Mi^T, Mr^T ]]           (W[p,i] layout, p = contraction)
with rhs columns stacked as [x_real(j=0..63); x_imag(j=0..63)] per `a`
slice, so out = W.T @ rhs gives [y_real(i); y_imag(i)] in one matmul
per 512-wide slice -- no PSUM accumulation.

The problem is HBM-bound (~358 GB/s per core); rel-err budget is 2e-2,
so all device I/O is bf16 (host casts fp32<->bf16): HBM traffic per
core drops from 67 MB to 33.5 MB.  Matmul runs bf16 x bf16 -> fp32
PSUM; the PSUM->SBUF downcast copies are split between the Vector and
Scalar engines (alternating per 1024-wide PSUM tile) so neither engine
is the critical path.  Store DMAs are issued from GpSimd (SWDGE) so a
store waiting on a cast never stalls a compute engine's FIFO; the
final two stores go out on Scalar/Sync HWDGE for a low-latency drain.
Measured end-to-end rel err ~3e-3.

Sharding: the leading `a` axis (untouched by the contraction) is split
8 ways -> 8 a-values per core; each core streams 16.8 MB in / 16.8 MB
out.
"""

import numpy as np
import ml_dtypes

import concourse.bacc as bacc
import concourse.mybir as mybir
import concourse.tile as tile
from concourse.bass_utils import run_bass_kernel_spmd

BF16 = ml_dtypes.bfloat16

DIM = 64
WIRES = 3
BATCH = 128
D = DIM**WIRES          # 262144
N_CORES = 8
A_PER_CORE = DIM // N_CORES     # 8 a-values per core
FREE = DIM * BATCH      # 8192 elements per (a, j) row
P = 128
MM_N = 512              # one matmul's fp32 PSUM output (one bank)
PS_N = 1024             # PSUM tile width (2 banks, 2 matmuls, 1 cast)

_cached = {}


def _build_nc():
    f32 = mybir.dt.float32
    bf16 = mybir.dt.bfloat16
    nc = bacc.Bacc("TRN2", target_bir_lowering=False, debug=False,
                   num_devices=N_CORES)
    xs = nc.dram_tensor("xs", [A_PER_CORE, P, FREE], bf16,
                        kind="ExternalInput").ap()
    w = nc.dram_tensor("w", [P, P], bf16, kind="ExternalInput").ap()
    ys = nc.dram_tensor("ys", [A_PER_CORE, P, FREE], bf16,
                        kind="ExternalOutput").ap()

    with tile.TileContext(nc) as tc:
        with (
            tc.tile_pool(name="wpool", bufs=1) as wpool,
            tc.tile_pool(name="inpool", bufs=4) as inpool,
            tc.tile_pool(name="outpool", bufs=4) as outpool,
            tc.tile_pool(name="pspool", bufs=4, space="PSUM") as pspool,
        ):
            wt = wpool.tile([P, P], bf16)
            # weight load off the Sync engine so the first bulk input
            # DMA issues as early as possible
            nc.gpsimd.dma_start(wt[:], w[:, :])

            # chunk schedule over the flattened (a, free) space: small
            # chunks at the start (compute/stores ramp up sooner) and at
            # the end (the last input chunk's matmul+cast+store pipeline
            # is the exposed tail), big 4K-element chunks in the middle.
            chunks = []  # (a, f0, fch)
            for a in range(A_PER_CORE):
                if a == 0:
                    split = [512, 512, 1024, 2048, 4096]
                elif a == A_PER_CORE - 1:
                    split = [4096, 2048, 1024, 512, 512]
                else:
                    split = [8192]
                f0 = 0
                for fch in split:
                    chunks.append((a, f0, fch))
                    f0 += fch
                assert f0 == FREE
            ncast = 0
            for ci, (a, f0, fch) in enumerate(chunks):
                last = ci == len(chunks) - 1
                xt = inpool.tile([P, fch], bf16, tag="xt")
                nc.sync.dma_start(xt[:], xs[a, :, f0:f0 + fch])
                yt = outpool.tile([P, fch], bf16, tag="yt")
                for t0 in range(0, fch, PS_N):
                    tn = min(PS_N, fch - t0)
                    ps = pspool.tile([P, PS_N], f32)
                    for k0 in range(0, tn, MM_N):
                        nc.tensor.matmul(ps[:, k0:k0 + MM_N], wt[:],
                                         xt[:, t0 + k0:t0 + k0 + MM_N],
                                         start=True, stop=True)
                    # alternate the PSUM->SBUF downcast between the two
                    # engines with a PSUM port (GpSimd has none); the
                    # final tile is split across both to halve the
                    # exposed cast latency in the drain chain.
                    if last and t0 + tn == fch:
                        h = tn // 2
                        nc.vector.tensor_copy(yt[:, t0:t0 + h], ps[:, :h])
                        nc.scalar.copy(yt[:, t0 + h:t0 + tn], ps[:, h:tn])
                    elif ncast % 2 == 0:
                        nc.vector.tensor_copy(yt[:, t0:t0 + tn], ps[:, :tn])
                    else:
                        nc.scalar.copy(yt[:, t0:t0 + tn], ps[:, :tn])
                    ncast += 1
                # SWDGE on GpSimd: the store's event-wait happens on the
                # Q7, not on an engine the cast pipeline needs.  Final
                # two stores drain via HWDGE (lower first-byte latency);
                # by then Scalar/Sync have no work left to block.
                if last:
                    nc.sync.dma_start(ys[a, :, f0:f0 + fch], yt[:])
                elif ci == len(chunks) - 2:
                    nc.scalar.dma_start(ys[a, :, f0:f0 + fch], yt[:])
                else:
                    nc.gpsimd.dma_start(ys[a, :, f0:f0 + fch], yt[:])

    nc.compile()
    return nc


def _get_nc():
    if "nc" not in _cached:
        _cached["nc"] = _build_nc()
    return _cached["nc"]


def kernel(M_real, M_imag, x_real, x_imag, **run_kwargs):
    M_real = np.ascontiguousarray(np.asarray(M_real, dtype=np.float32))
    M_imag = np.ascontiguousarray(np.asarray(M_imag, dtype=np.float32))
    x_real = np.asarray(x_real, dtype=np.float32)
    x_imag = np.asarray(x_imag, dtype=np.float32)

    # Stationary weight W[p, i] (see module docstring), bf16 on device.
    W = np.block([[M_real.T, M_imag.T],
                  [-M_imag.T, M_real.T]]).astype(BF16)
    W = np.ascontiguousarray(W)

    # Interleave real/imag along the partition axis: xs[a, 0:64, f] = real,
    # xs[a, 64:128, f] = imag, with f = r*128 + b.  Cast fp32 -> bf16.
    xs_all = np.empty((DIM, P, FREE), dtype=BF16)
    xs_all[:, :DIM, :] = x_real.reshape(DIM, DIM, FREE)
    xs_all[:, DIM:, :] = x_imag.reshape(DIM, DIM, FREE)

    nc = _get_nc()
    in_maps = [
        {"xs": xs_all[c * A_PER_CORE:(c + 1) * A_PER_CORE], "w": W}
        for c in range(N_CORES)
    ]
    r = run_bass_kernel_spmd(nc, in_maps, list(range(N_CORES)), **run_kwargs)
    if run_kwargs:
        _cached["last_result"] = r
    results = r.results

    ys_all = np.concatenate([results[c]["ys"] for c in range(N_CORES)], axis=0)
    out = np.empty((D, BATCH), dtype=np.complex64)
    out.real = ys_all[:, :DIM, :].astype(np.float32).reshape(D, BATCH)
    out.imag = ys_all[:, DIM:, :].astype(np.float32).reshape(D, BATCH)
    return out


# revision 6
# speedup vs baseline: 1.0013x; 1.0013x over previous
"""Trainium2 Bass kernel for nn_CustomGate: y = (I_64 (x) M (x) I_64) @ x.

Math: viewing x as (a=64, j=64, r=64, b=128), the gate is
    y[a,i,r,b] = sum_j M[i,j] * x[a,j,r,b]      (complex, M is 64x64)

Complex arithmetic is folded into one real 128x128 stationary weight
    W = [[Mr^T,  Mi^T ],
         [-Mi^T, Mr^T ]]           (W[p,i] layout, p = contraction)
with rhs columns stacked as [x_real(j=0..63); x_imag(j=0..63)] per `a`
slice, so out = W.T @ rhs gives [y_real(i); y_imag(i)] in one matmul
per 512-wide slice -- no PSUM accumulation.

The problem is HBM-bound (~358 GB/s per core); rel-err budget is 2e-2,
so all device I/O is bf16 (host casts fp32<->bf16): HBM traffic per
core drops from 67 MB to 33.5 MB.  Matmul runs bf16 x bf16 -> fp32
PSUM; the PSUM->SBUF downcast copies are split between the Vector and
Scalar engines (alternating per 1024-wide PSUM tile) so neither engine
is the critical path.  Store DMAs are issued from GpSimd (SWDGE) so a
store waiting on a cast never stalls a compute engine's FIFO; the
final two stores go out on Scalar/Sync HWDGE for a low-latency drain.
Measured end-to-end rel err ~3e-3.

Sharding: the leading `a` axis (untouched by the contraction) is split
8 ways -> 8 a-values per core; each core streams 16.8 MB in / 16.8 MB
out.
"""

import numpy as np
import ml_dtypes

import concourse.bacc as bacc
import concourse.mybir as mybir
import concourse.tile as tile
from concourse.bass_utils import run_bass_kernel_spmd

BF16 = ml_dtypes.bfloat16

DIM = 64
WIRES = 3
BATCH = 128
D = DIM**WIRES          # 262144
N_CORES = 8
A_PER_CORE = DIM // N_CORES     # 8 a-values per core
FREE = DIM * BATCH      # 8192 elements per (a, j) row
P = 128
MM_N = 512              # one matmul's fp32 PSUM output (one bank)
PS_N = 1024             # PSUM tile width (2 banks, 2 matmuls, 1 cast)

_cached = {}


def _build_nc():
    f32 = mybir.dt.float32
    bf16 = mybir.dt.bfloat16
    nc = bacc.Bacc("TRN2", target_bir_lowering=False, debug=False,
                   num_devices=N_CORES)
    xs = nc.dram_tensor("xs", [A_PER_CORE, P, FREE], bf16,
                        kind="ExternalInput").ap()
    w = nc.dram_tensor("w", [P, P], bf16, kind="ExternalInput").ap()
    ys = nc.dram_tensor("ys", [A_PER_CORE, P, FREE], bf16,
                        kind="ExternalOutput").ap()

    with tile.TileContext(nc) as tc:
        with (
            tc.tile_pool(name="wpool", bufs=1) as wpool,
            tc.tile_pool(name="inpool", bufs=4) as inpool,
            tc.tile_pool(name="outpool", bufs=4) as outpool,
            tc.tile_pool(name="pspool", bufs=4, space="PSUM") as pspool,
        ):
            wt = wpool.tile([P, P], bf16)
            # weight load off the Sync engine so the first bulk input
            # DMA issues as early as possible
            nc.gpsimd.dma_start(wt[:], w[:, :])

            # chunk schedule over the flattened (a, free) space: small
            # chunks at the start (compute/stores ramp up sooner) and at
            # the end (the last input chunk's matmul+cast+store pipeline
            # is the exposed tail), big 4K-element chunks in the middle.
            chunks = []  # (a, f0, fch)
            for a in range(A_PER_CORE):
                if a == 0:
                    split = [512, 512, 1024, 2048, 4096]
                elif a == A_PER_CORE - 1:
                    split = [4096, 2048, 1024, 512, 512]
                else:
                    split = [4096, 4096]
                f0 = 0
                for fch in split:
                    chunks.append((a, f0, fch))
                    f0 += fch
                assert f0 == FREE
            ncast = 0
            for ci, (a, f0, fch) in enumerate(chunks):
                last = ci == len(chunks) - 1
                xt = inpool.tile([P, fch], bf16, tag="xt")
                nc.sync.dma_start(xt[:], xs[a, :, f0:f0 + fch])
                yt = outpool.tile([P, fch], bf16, tag="yt")
                for t0 in range(0, fch, PS_N):
                    tn = min(PS_N, fch - t0)
                    ps = pspool.tile([P, PS_N], f32)
                    for k0 in range(0, tn, MM_N):
                        nc.tensor.matmul(ps[:, k0:k0 + MM_N], wt[:],
                                         xt[:, t0 + k0:t0 + k0 + MM_N],
                                         start=True, stop=True)
                    # alternate the PSUM->SBUF downcast between the two
                    # engines with a PSUM port (GpSimd has none); the
                    # final tile is split across both to halve the
                    # exposed cast latency in the drain chain.
                    if last and t0 + tn == fch:
                        h = tn // 2
                        nc.vector.tensor_copy(yt[:, t0:t0 + h], ps[:, :h])
                        nc.scalar.copy(yt[:, t0 + h:t0 + tn], ps[:, h:tn])
                    elif ncast % 2 == 0:
                        nc.vector.tensor_copy(yt[:, t0:t0 + tn], ps[:, :tn])
                    else:
                        nc.scalar.copy(yt[:, t0:t0 + tn], ps[:, :tn])
                    ncast += 1
                # SWDGE on GpSimd: the store's event-wait happens on the
                # Q7, not on an engine the cast pipeline needs.  Final
                # two stores drain via HWDGE (lower first-byte latency);
                # by then Scalar/Sync have no work left to block.
                if last:
                    nc.sync.dma_start(ys[a, :, f0:f0 + fch], yt[:])
                elif ci == len(chunks) - 2:
                    nc.scalar.dma_start(ys[a, :, f0:f0 + fch], yt[:])
                else:
                    nc.gpsimd.dma_start(ys[a, :, f0:f0 + fch], yt[:])

    nc.compile()
    return nc


def _get_nc():
    if "nc" not in _cached:
        _cached["nc"] = _build_nc()
    return _cached["nc"]


def kernel(M_real, M_imag, x_real, x_imag, **run_kwargs):
    M_real = np.ascontiguousarray(np.asarray(M_real, dtype=np.float32))
    M_imag = np.ascontiguousarray(np.asarray(M_imag, dtype=np.float32))
    x_real = np.asarray(x_real, dtype=np.float32)
    x_imag = np.asarray(x_imag, dtype=np.float32)

    # Stationary weight W[p, i] (see module docstring), bf16 on device.
    W = np.block([[M_real.T, M_imag.T],
                  [-M_imag.T, M_real.T]]).astype(BF16)
    W = np.ascontiguousarray(W)

    # Interleave real/imag along the partition axis: xs[a, 0:64, f] = real,
    # xs[a, 64:128, f] = imag, with f = r*128 + b.  Cast fp32 -> bf16.
    xs_all = np.empty((DIM, P, FREE), dtype=BF16)
    xs_all[:, :DIM, :] = x_real.reshape(DIM, DIM, FREE)
    xs_all[:, DIM:, :] = x_imag.reshape(DIM, DIM, FREE)

    nc = _get_nc()
    in_maps = [
        {"xs": xs_all[c * A_PER_CORE:(c + 1) * A_PER_CORE], "w": W}
        for c in range(N_CORES)
    ]
    r = run_bass_kernel_spmd(nc, in_maps, list(range(N_CORES)), **run_kwargs)
    if run_kwargs:
        _cached["last_result"] = r
    results = r.results

    ys_all = np.concatenate([results[c]["ys"] for c in range(N_CORES)], axis=0)
    out = np.empty((D, BATCH), dtype=np.complex64)
    out.real = ys_all[:, :DIM, :].astype(np.float32).reshape(D, BATCH)
    out.imag = ys_all[:, DIM:, :].astype(np.float32).reshape(D, BATCH)
    return out
